# revision 7
# baseline (speedup 1.0000x reference)
"""Causal multi-head attention (B=4, T=2048, D=1024, H=16) on 8 TRN2 NeuronCores.

Sharding: core c -> batch b = c // 2, head-group g = c % 2 (8 heads each).
Host pre-transposes x to x^T per batch and pre-slices W_qkv/W_o/biases per
head-group (1/sqrt(dh) folded into W_q/b_q on host).  Each core:

  phase 1: Q^T,K^T  (qkv^T layout, [dh, t], heads pair-stacked on partitions)
           V        (natural [t, dh] layout, ones-augmented for row-sums)
  phase 2: per head-pair, per 512-wide q-chunk, per 128-wide k-tile:
           S^T = K^T.T Q^T (two heads row-packed into one 2-bank psum tile),
           exp via ScalarE -> P^T (bf16), causal via suffix-trimming +
           triangular mask multiply on diagonal tiles,
           o^T (+rowsum) = V_aug.T @ P^T accumulated in psum,
           normalize via DVE reciprocal + rank-1 ones broadcast matmul.
  phase 3: partial out = o_norm^T.T @ W_o  ->  DRAM.

Host sums the two head-group partials per batch and adds b_o.

Matmul dtypes: fp32r (TF32-class, full PE rate at N>=256) for projections,
bf16 for score/AV stages (SBUF capacity).  A post-scheduling pass splits
multi-semaphore waits (walrus allows only one sync-wait on several ISA
structs, e.g. the fused fp32 weight-load).
"""

import sys

sys.path.insert(0, "/opt/trn_rl_repo")

import numpy as np

import concourse.bass as bass
import concourse.mybir as mybir
from concourse.bass_utils import run_bass_kernel_spmd
from concourse.tile import TileContext

F32 = mybir.dt.float32
F32R = mybir.dt.float32r
BF16 = mybir.dt.bfloat16
EXP = mybir.ActivationFunctionType.Exp

B, T, D, H = 4, 2048, 1024, 16
DH = D // H          # 64
HPC = H // 2         # heads per core = 8
DPC = HPC * DH       # 512 projected dims per core
N_CORES = 8
QC = 512             # q-chunk width in phase 2
KT = 128             # k-tile width


def split_excess_waits(nc, cap=1):
    """walrus limits sync-wait slots per ISA instruction (1 for several
    structs).  Move excess waits onto InstEventSemaphore instructions
    inserted just before the offender on the same engine."""
    n_split = 0
    for f in nc.m.functions:
        for blk in f.blocks:
            insts = blk.instructions
            out = []
            changed = False
            for inst in insts:
                si = inst.sync_info
                waits = list(si.on_wait) if si is not None else []
                if len(waits) > cap:
                    for j, w in enumerate(waits[:-cap]):
                        ev = mybir.InstEventSemaphore(
                            name=f"{inst.name}-w{j}", ins=[], outs=[]
                        )
                        ev.engine = inst.engine
                        ev.sync_info = mybir.SyncInfo(on_wait=[w], on_update=[])
                        out.append(ev)
                        n_split += 1
                    inst.sync_info = mybir.SyncInfo(
                        on_wait=waits[-cap:], on_update=list(si.on_update)
                    )
                    changed = True
                out.append(inst)
            if changed:
                blk.instructions = out
    return n_split


def build():
    nc = bass.Bass(target_bir_lowering=False)

    xT_d = nc.dram_tensor("xT", [D, T], F32, kind="ExternalInput")
    wqk_d = nc.dram_tensor("wqk", [D, 2 * DPC], F32, kind="ExternalInput")
    wv_d = nc.dram_tensor("wv", [D, DPC], F32, kind="ExternalInput")
    wo_d = nc.dram_tensor("wo", [DPC, D], F32, kind="ExternalInput")
    bqk_d = nc.dram_tensor("bqk", [128, 8], F32, kind="ExternalInput")
    bv_d = nc.dram_tensor("bv", [1, DPC], F32, kind="ExternalInput")
    mask_d = nc.dram_tensor("trimask", [128, 128], BF16, kind="ExternalInput")
    ones_d = nc.dram_tensor("ones", [1, 128], F32, kind="ExternalInput")
    out_d = nc.dram_tensor("out", [T, D], F32, kind="ExternalOutput")

    with TileContext(nc) as tc:
        with (
            tc.tile_pool(name="const", bufs=1) as constp,
            tc.tile_pool(name="wstream", bufs=2) as wp,
            tc.tile_pool(name="xt", bufs=1) as xtp,
            tc.tile_pool(name="qk", bufs=1) as qkp,
            tc.tile_pool(name="vaug", bufs=1) as vp,
            tc.tile_pool(name="onorm", bufs=1) as onp,
            tc.tile_pool(name="pt", bufs=3) as ptp,
            tc.tile_pool(name="small", bufs=4) as smallp,
            tc.tile_pool(name="osb", bufs=3) as osbp,
            tc.tile_pool(name="ps", bufs=2, space="PSUM") as psp,
            tc.tile_pool(name="spair", bufs=2, space="PSUM") as spp,
            tc.tile_pool(name="oacc", bufs=2, space="PSUM") as oap,
        ):
            # ---- constants ----
            wv_sb = constp.tile([128, 8, DPC], F32R, tag="wv")
            nc.sync.dma_start(
                wv_sb[:], wv_d[:].rearrange("(dt p) c -> p dt c", p=128).bitcast(F32R)
            )
            wo_sb = constp.tile([128, 4, D], F32R, tag="wo")
            nc.sync.dma_start(
                wo_sb[:], wo_d[:].rearrange("(hp p) c -> p hp c", p=128).bitcast(F32R)
            )
            bqk_sb = constp.tile([128, 8], F32, tag="bqk")
            nc.sync.dma_start(bqk_sb[:], bqk_d[:])
            bv_sb = constp.tile([1, DPC], F32R, tag="bv")
            nc.sync.dma_start(bv_sb[:], bv_d[:].bitcast(F32R))
            mask_sb = constp.tile([128, 128], BF16, tag="mask")
            nc.sync.dma_start(mask_sb[:], mask_d[:])
            ones128 = constp.tile([1, 128], F32R, tag="ones128")
            nc.sync.dma_start(ones128[:], ones_d[:].bitcast(F32R))
            ones64 = ones128[:, 0:64]

            # persistent activations
            qk_sb = [qkp.tile([128, T], BF16, tag=f"qk{j}", name=f"qk{j}") for j in range(8)]
            vaug = [vp.tile([128, HPC, DH + 1], BF16, tag=f"v{t}", name=f"v{t}") for t in range(16)]
            onorm = [onp.tile([128, T], F32R, tag=f"on{hp}", name=f"on{hp}") for hp in range(4)]

            # ---- phase 1: projections, in two t-halves ----
            for th in range(2):
                t0 = th * (T // 2)
                xt = []
                for dt in range(8):
                    x_t = xtp.tile([128, T // 2], F32R, tag=f"xt{dt}")
                    nc.sync.dma_start(
                        x_t[:],
                        xT_d[128 * dt : 128 * (dt + 1), t0 : t0 + T // 2].bitcast(F32R),
                    )
                    xt.append(x_t)

                # Q^T / K^T:  [d', t] = W[:, d'].T @ x^T
                for j in range(8):
                    w_j = wp.tile([128, 8, 128], F32R, tag="wqk")
                    nc.sync.dma_start(
                        w_j[:],
                        wqk_d[:, 128 * j : 128 * (j + 1)]
                        .rearrange("(dt p) c -> p dt c", p=128)
                        .bitcast(F32R),
                    )
                    for tc_ in range(2):
                        ps = psp.tile([128, 512], F32, tag="ps")
                        for dt in range(8):
                            nc.tensor.matmul(
                                ps[:],
                                w_j[:, dt, :],
                                xt[dt][:, 512 * tc_ : 512 * (tc_ + 1)],
                                start=(dt == 0),
                                stop=(dt == 7),
                            )
                        nc.vector.tensor_scalar_add(
                            qk_sb[j][:, t0 + 512 * tc_ : t0 + 512 * (tc_ + 1)],
                            ps[:],
                            bqk_sb[:, j : j + 1],
                        )

                # V (natural layout), ones-augmented
                for tt in range(8):
                    tg = th * 8 + tt
                    ps = psp.tile([128, 512], F32, tag="ps")
                    for dt in range(8):
                        nc.tensor.matmul(
                            ps[:],
                            xt[dt][:, 128 * tt : 128 * (tt + 1)],
                            wv_sb[:, dt, :],
                            start=(dt == 0),
                            stop=False,
                        )
                    nc.tensor.matmul(
                        ps[:], ones128[:], bv_sb[:], start=False, stop=True
                    )
                    nc.vector.tensor_copy(
                        out=vaug[tg][:, :, 0:DH],
                        in_=ps[:].rearrange("p (h d) -> p h d", h=HPC),
                    )
                    nc.vector.memset(vaug[tg][:, :, DH : DH + 1], 1.0)

            # ---- phase 2: attention per head pair ----
            for hp in range(4):
                qT = qk_sb[hp]
                kT = qk_sb[4 + hp]
                for c in range(4):
                    q0 = QC * c
                    ktiles = 4 * (c + 1)
                    oA = oap.tile([128, 512], F32, tag="oacc")
                    oB = oap.tile([128, 512], F32, tag="oacc")
                    for t in range(ktiles):
                        j = t - 4 * c
                        qs = 128 * j if j >= 0 else 0
                        sp = spp.tile([128, 1024], F32, tag="sp")
                        for half, base in ((0, 0), (1, 64)):
                            nc.tensor.matmul(
                                sp[:, 512 * half + qs : 512 * (half + 1)],
                                kT[base : base + 64, 128 * t : 128 * (t + 1)],
                                qT[base : base + 64, q0 + qs : q0 + QC],
                                start=True,
                                stop=True,
                                tile_position=(base, 0),
                            )
                        pt = ptp.tile([128, 1024], BF16, tag="pt")
                        if qs == 0:
                            nc.scalar.activation(pt[:], sp[:], EXP)
                        else:
                            nc.scalar.activation(
                                pt[:, qs:512], sp[:, qs:512], EXP
                            )
                            nc.scalar.activation(
                                pt[:, 512 + qs : 1024], sp[:, 512 + qs : 1024], EXP
                            )
                        if j >= 0:
                            for half in (0, 1):
                                lo = 512 * half + qs
                                nc.vector.tensor_tensor(
                                    pt[:, lo : lo + 128],
                                    pt[:, lo : lo + 128],
                                    mask_sb[:],
                                    mybir.AluOpType.mult,
                                )
                        for o_ps, half in ((oA, 0), (oB, 1)):
                            nc.tensor.matmul(
                                o_ps[0 : DH + 1, qs:512],
                                vaug[t][:, 2 * hp + half, :],
                                pt[:, 512 * half + qs : 512 * (half + 1)],
                                start=(t == 0),
                                stop=(t == ktiles - 1),
                                skip_group_check=True,
                            )
                    # normalize both heads of the pair
                    for o_ps, base in ((oA, 0), (oB, 64)):
                        rt = smallp.tile([1, 512], F32R, tag="rt")
                        with nc.allow_low_precision(reason="softmax denom"):
                            nc.vector.reciprocal(rt[:], o_ps[DH : DH + 1, :])
                        bc_ps = psp.tile([128, 512], F32, tag="ps")
                        nc.tensor.matmul(
                            bc_ps[0:64, :], ones64, rt[:], start=True, stop=True
                        )
                        bc_sb = smallp.tile([64, 512], F32, tag="bc")
                        nc.vector.tensor_copy(out=bc_sb[:], in_=bc_ps[0:64, :])
                        nc.vector.tensor_tensor(
                            onorm[hp][base : base + 64, q0 : q0 + QC],
                            o_ps[0:DH, :],
                            bc_sb[:],
                            mybir.AluOpType.mult,
                        )

            # ---- phase 3: output projection ----
            for qt in range(16):
                for dc in range(2):
                    ps = psp.tile([128, 512], F32, tag="ps")
                    for hp in range(4):
                        nc.tensor.matmul(
                            ps[:],
                            onorm[hp][:, 128 * qt : 128 * (qt + 1)],
                            wo_sb[:, hp, 512 * dc : 512 * (dc + 1)],
                            start=(hp == 0),
                            stop=(hp == 3),
                        )
                    osb = osbp.tile([128, 512], F32, tag="osb")
                    nc.vector.tensor_copy(out=osb[:], in_=ps[:])
                    nc.sync.dma_start(
                        out_d[128 * qt : 128 * (qt + 1), 512 * dc : 512 * (dc + 1)],
                        osb[:],
                    )

    split_excess_waits(nc)
    return nc


TRACE = False
LAST_EXEC_NS = None

_NC = None


def _get_nc():
    global _NC
    if _NC is None:
        _NC = build()
    return _NC


def kernel(x, W_qkv, b_qkv, W_o, b_o):
    x = np.asarray(x, dtype=np.float32)
    W_qkv = np.asarray(W_qkv, dtype=np.float32)
    b_qkv = np.asarray(b_qkv, dtype=np.float32)
    W_o = np.asarray(W_o, dtype=np.float32)
    b_o = np.asarray(b_o, dtype=np.float32)
    import ml_dtypes

    scale = 1.0 / np.sqrt(np.float32(DH))

    # x^T per batch (shared between the two cores of a batch)
    xTs = [np.ascontiguousarray(x[b].T) for b in range(B)]

    # causal mask tile: keep iff q-local >= k-local (upper triangular w/ diag)
    tri = np.triu(np.ones((128, 128), np.float32)).astype(ml_dtypes.bfloat16)

    in_maps = []
    for c in range(N_CORES):
        b, g = divmod(c, 2)
        h0 = g * HPC
        qcols = slice(h0 * DH, h0 * DH + DPC)
        kcols = slice(D + h0 * DH, D + h0 * DH + DPC)
        vcols = slice(2 * D + h0 * DH, 2 * D + h0 * DH + DPC)
        wqk = np.concatenate(
            [W_qkv[:, qcols] * scale, W_qkv[:, kcols]], axis=1
        ).astype(np.float32)
        bqk = np.concatenate(
            [b_qkv[qcols] * scale, b_qkv[kcols]]
        ).astype(np.float32)
        in_maps.append(
            {
                "xT": xTs[b],
                "wqk": np.ascontiguousarray(wqk),
                "wv": np.ascontiguousarray(W_qkv[:, vcols]),
                "wo": np.ascontiguousarray(W_o[g * DPC : (g + 1) * DPC, :]),
                "bqk": np.ascontiguousarray(bqk.reshape(8, 128).T),
                "bv": np.ascontiguousarray(b_qkv[vcols].reshape(1, DPC)),
                "trimask": tri,
                "ones": np.ones((1, 128), np.float32),
            }
        )

    nc = _get_nc()
    global LAST_EXEC_NS
    res = run_bass_kernel_spmd(nc, in_maps, list(range(N_CORES)), trace=TRACE)
    LAST_EXEC_NS = res.exec_time_ns
    LAST_RES = globals().setdefault("_LAST_RES", None)
    globals()["_LAST_RES"] = res
    parts = [res.results[c]["out"] for c in range(N_CORES)]
    out = np.empty((B, T, D), np.float32)
    for b in range(B):
        out[b] = parts[2 * b] + parts[2 * b + 1] + b_o[None, :]
    return out


# revision 8
# speedup vs baseline: 1.0014x; 1.0014x over previous
"""Causal multi-head attention (B=4, T=2048, D=1024, H=16) on 8 TRN2 NeuronCores.

Sharding: core c -> batch b = c // 2, head-group g = c % 2 (8 heads each).
Host pre-transposes x to x^T per batch and pre-slices W_qkv/W_o/biases per
head-group (1/sqrt(dh) folded into W_q/b_q on host).  Each core:

  phase 1: Q^T,K^T  (qkv^T layout, [dh, t], heads pair-stacked on partitions)
           V        (natural [t, dh] layout, ones-augmented for row-sums)
  phase 2: per head-pair, per 512-wide q-chunk, per 128-wide k-tile:
           S^T = K^T.T Q^T (two heads row-packed into one 2-bank psum tile),
           exp via ScalarE -> P^T (bf16), causal via suffix-trimming +
           triangular mask multiply on diagonal tiles,
           o^T (+rowsum) = V_aug.T @ P^T accumulated in psum,
           normalize via DVE reciprocal + rank-1 ones broadcast matmul.
  phase 3: partial out = o_norm^T.T @ W_o  ->  DRAM.

Host sums the two head-group partials per batch and adds b_o.

Matmul dtypes: fp32r (TF32-class, full PE rate at N>=256) for projections,
bf16 for score/AV stages (SBUF capacity).  A post-scheduling pass splits
multi-semaphore waits (walrus allows only one sync-wait on several ISA
structs, e.g. the fused fp32 weight-load).
"""

import sys

sys.path.insert(0, "/opt/trn_rl_repo")

import numpy as np

import concourse.bass as bass
import concourse.mybir as mybir
from concourse.bass_utils import run_bass_kernel_spmd
from concourse.tile import TileContext

F32 = mybir.dt.float32
F32R = mybir.dt.float32r
BF16 = mybir.dt.bfloat16
EXP = mybir.ActivationFunctionType.Exp

B, T, D, H = 4, 2048, 1024, 16
DH = D // H          # 64
HPC = H // 2         # heads per core = 8
DPC = HPC * DH       # 512 projected dims per core
N_CORES = 8
QC = 512             # q-chunk width in phase 2
KT = 128             # k-tile width


def split_excess_waits(nc, cap=1):
    """walrus limits sync-wait slots per ISA instruction (1 for several
    structs).  Move excess waits onto InstEventSemaphore instructions
    inserted just before the offender on the same engine."""
    n_split = 0
    for f in nc.m.functions:
        for blk in f.blocks:
            insts = blk.instructions
            out = []
            changed = False
            for inst in insts:
                si = inst.sync_info
                waits = list(si.on_wait) if si is not None else []
                if len(waits) > cap:
                    for j, w in enumerate(waits[:-cap]):
                        ev = mybir.InstEventSemaphore(
                            name=f"{inst.name}-w{j}", ins=[], outs=[]
                        )
                        ev.engine = inst.engine
                        ev.sync_info = mybir.SyncInfo(on_wait=[w], on_update=[])
                        out.append(ev)
                        n_split += 1
                    inst.sync_info = mybir.SyncInfo(
                        on_wait=waits[-cap:], on_update=list(si.on_update)
                    )
                    changed = True
                out.append(inst)
            if changed:
                blk.instructions = out
    return n_split


def build():
    nc = bass.Bass(target_bir_lowering=False)

    xT_d = nc.dram_tensor("xT", [D, T], F32, kind="ExternalInput")
    wqk_d = nc.dram_tensor("wqk", [D, 2 * DPC], F32, kind="ExternalInput")
    wv_d = nc.dram_tensor("wv", [D, DPC], F32, kind="ExternalInput")
    wo_d = nc.dram_tensor("wo", [DPC, D], F32, kind="ExternalInput")
    bqk_d = nc.dram_tensor("bqk", [128, 8], F32, kind="ExternalInput")
    bv_d = nc.dram_tensor("bv", [1, DPC], F32, kind="ExternalInput")
    mask_d = nc.dram_tensor("trimask", [128, 256], BF16, kind="ExternalInput")
    ones_d = nc.dram_tensor("ones", [1, 128], F32, kind="ExternalInput")
    out_d = nc.dram_tensor("out", [T, D], F32, kind="ExternalOutput")

    with TileContext(nc) as tc:
        with (
            tc.tile_pool(name="const", bufs=1) as constp,
            tc.tile_pool(name="wstream", bufs=2) as wp,
            tc.tile_pool(name="xt", bufs=1) as xtp,
            tc.tile_pool(name="qk", bufs=1) as qkp,
            tc.tile_pool(name="vaug", bufs=1) as vp,
            tc.tile_pool(name="onorm", bufs=1) as onp,
            tc.tile_pool(name="pt", bufs=4) as ptp,
            tc.tile_pool(name="small", bufs=4) as smallp,
            tc.tile_pool(name="osb", bufs=3) as osbp,
            tc.tile_pool(name="ps", bufs=2, space="PSUM") as psp,
            tc.tile_pool(name="spair", bufs=2, space="PSUM") as spp,
            tc.tile_pool(name="oacc", bufs=2, space="PSUM") as oap,
        ):
            # ---- constants ----
            wv_sb = constp.tile([128, 8, DPC], F32R, tag="wv")
            nc.sync.dma_start(
                wv_sb[:], wv_d[:].rearrange("(dt p) c -> p dt c", p=128).bitcast(F32R)
            )
            wo_sb = constp.tile([128, 4, D], F32R, tag="wo")
            nc.sync.dma_start(
                wo_sb[:], wo_d[:].rearrange("(hp p) c -> p hp c", p=128).bitcast(F32R)
            )
            bqk_sb = constp.tile([128, 8], F32, tag="bqk")
            nc.sync.dma_start(bqk_sb[:], bqk_d[:])
            bv_sb = constp.tile([1, DPC], F32R, tag="bv")
            nc.sync.dma_start(bv_sb[:], bv_d[:].bitcast(F32R))
            mask_sb = constp.tile([128, 2, 128], BF16, tag="mask")
            nc.sync.dma_start(mask_sb[:], mask_d[:].rearrange("p (h q) -> p h q", h=2))
            ones128 = constp.tile([1, 128], F32R, tag="ones128")
            nc.sync.dma_start(ones128[:], ones_d[:].bitcast(F32R))
            ones64 = ones128[:, 0:64]

            # persistent activations
            qk_sb = [qkp.tile([128, T], BF16, tag=f"qk{j}", name=f"qk{j}") for j in range(8)]
            vaug = [vp.tile([128, HPC, DH + 1], BF16, tag=f"v{t}", name=f"v{t}") for t in range(16)]
            onorm = [onp.tile([128, T], F32R, tag=f"on{hp}", name=f"on{hp}") for hp in range(4)]

            # ---- phase 1: projections, in two t-halves ----
            def load_wj(th, j):
                w_j = wp.tile([128, 8, 128], F32R, tag="wqk", name=f"w{th}_{j}")
                nc.sync.dma_start(
                    w_j[:],
                    wqk_d[:, 128 * j : 128 * (j + 1)]
                    .rearrange("(dt p) c -> p dt c", p=128)
                    .bitcast(F32R),
                )
                return w_j

            for th in range(2):
                t0 = th * (T // 2)
                w_first = load_wj(th, 0) if th == 0 else None
                xt = []
                for dt in range(8):
                    x_t = xtp.tile([128, T // 2], F32R, tag=f"xt{dt}")
                    nc.sync.dma_start(
                        x_t[:],
                        xT_d[128 * dt : 128 * (dt + 1), t0 : t0 + T // 2].bitcast(F32R),
                    )
                    xt.append(x_t)

                # Q^T / K^T:  [d', t] = W[:, d'].T @ x^T
                for j in range(8):
                    w_j = w_first if (th == 0 and j == 0) else load_wj(th, j)
                    for tc_ in range(2):
                        ps = psp.tile([128, 512], F32, tag="ps")
                        for dt in range(8):
                            nc.tensor.matmul(
                                ps[:],
                                w_j[:, dt, :],
                                xt[dt][:, 512 * tc_ : 512 * (tc_ + 1)],
                                start=(dt == 0),
                                stop=(dt == 7),
                            )
                        nc.vector.tensor_scalar_add(
                            qk_sb[j][:, t0 + 512 * tc_ : t0 + 512 * (tc_ + 1)],
                            ps[:],
                            bqk_sb[:, j : j + 1],
                        )

                # V (natural layout), ones-augmented
                for tt in range(8):
                    tg = th * 8 + tt
                    ps = psp.tile([128, 512], F32, tag="ps")
                    for dt in range(8):
                        nc.tensor.matmul(
                            ps[:],
                            xt[dt][:, 128 * tt : 128 * (tt + 1)],
                            wv_sb[:, dt, :],
                            start=(dt == 0),
                            stop=False,
                        )
                    nc.tensor.matmul(
                        ps[:], ones128[:], bv_sb[:], start=False, stop=True
                    )
                    nc.vector.tensor_copy(
                        out=vaug[tg][:, :, 0:DH],
                        in_=ps[:].rearrange("p (h d) -> p h d", h=HPC),
                    )
                    nc.gpsimd.memset(vaug[tg][:, :, DH : DH + 1], 1.0)

            # ---- phase 2: attention per head pair ----
            for hp in range(4):
                qT = qk_sb[hp]
                kT = qk_sb[4 + hp]
                for c in range(4):
                    q0 = QC * c
                    ktiles = 4 * (c + 1)
                    oA = oap.tile([128, 512], F32, tag="oacc")
                    oB = oap.tile([128, 512], F32, tag="oacc")
                    for t in range(ktiles):
                        j = t - 4 * c
                        qs = 128 * j if j >= 0 else 0
                        sp = spp.tile([128, 1024], F32, tag="sp")
                        for half, base in ((0, 0), (1, 64)):
                            nc.tensor.matmul(
                                sp[:, 512 * half + qs : 512 * (half + 1)],
                                kT[base : base + 64, 128 * t : 128 * (t + 1)],
                                qT[base : base + 64, q0 + qs : q0 + QC],
                                start=True,
                                stop=True,
                                tile_position=(base, 0),
                            )
                        pt = ptp.tile([128, 1024], BF16, tag="pt")
                        if qs == 0:
                            nc.scalar.activation(pt[:], sp[:], EXP)
                        else:
                            nc.scalar.activation(
                                pt[:, qs:512], sp[:, qs:512], EXP
                            )
                            nc.scalar.activation(
                                pt[:, 512 + qs : 1024], sp[:, 512 + qs : 1024], EXP
                            )
                        if j >= 0:
                            ptv = pt[:].rearrange("p (h q) -> p h q", h=2)[
                                :, :, qs : qs + 128
                            ]
                            nc.gpsimd.tensor_tensor(
                                ptv, ptv, mask_sb[:], mybir.AluOpType.mult
                            )
                        for o_ps, half in ((oA, 0), (oB, 1)):
                            nc.tensor.matmul(
                                o_ps[0 : DH + 1, qs:512],
                                vaug[t][:, 2 * hp + half, :],
                                pt[:, 512 * half + qs : 512 * (half + 1)],
                                start=(t == 0),
                                stop=(t == ktiles - 1),
                                skip_group_check=True,
                            )
                    # normalize both heads: copy o^T to SBUF first so the
                    # psum accumulator slot frees immediately, then
                    # recip -> rank-1 broadcast -> multiply (from psum bcast).
                    for o_ps, base in ((oA, 0), (oB, 64)):
                        ob = smallp.tile([128, 512], F32, tag="ob")
                        nc.vector.tensor_copy(
                            out=ob[0 : DH + 1, :], in_=o_ps[0 : DH + 1, :]
                        )
                        rt = smallp.tile([1, 512], F32R, tag="rt")
                        with nc.allow_low_precision(reason="softmax denom"):
                            nc.vector.reciprocal(rt[:], ob[DH : DH + 1, :])
                        bc_ps = psp.tile([128, 512], F32, tag="ps")
                        nc.tensor.matmul(
                            bc_ps[0:64, :], ones64, rt[:], start=True, stop=True
                        )
                        nc.vector.tensor_tensor(
                            onorm[hp][base : base + 64, q0 : q0 + QC],
                            ob[0:DH, :],
                            bc_ps[0:64, :],
                            mybir.AluOpType.mult,
                        )

            # ---- phase 3: output projection ----
            for qt in range(16):
                for dc in range(2):
                    ps = psp.tile([128, 512], F32, tag="ps")
                    for hp in range(4):
                        nc.tensor.matmul(
                            ps[:],
                            onorm[hp][:, 128 * qt : 128 * (qt + 1)],
                            wo_sb[:, hp, 512 * dc : 512 * (dc + 1)],
                            start=(hp == 0),
                            stop=(hp == 3),
                        )
                    osb = osbp.tile([128, 512], F32, tag="osb")
                    nc.vector.tensor_copy(out=osb[:], in_=ps[:])
                    nc.sync.dma_start(
                        out_d[128 * qt : 128 * (qt + 1), 512 * dc : 512 * (dc + 1)],
                        osb[:],
                    )

    split_excess_waits(nc)
    return nc


TRACE = False
LAST_EXEC_NS = None

_NC = None


def _get_nc():
    global _NC
    if _NC is None:
        _NC = build()
    return _NC


def kernel(x, W_qkv, b_qkv, W_o, b_o):
    x = np.asarray(x, dtype=np.float32)
    W_qkv = np.asarray(W_qkv, dtype=np.float32)
    b_qkv = np.asarray(b_qkv, dtype=np.float32)
    W_o = np.asarray(W_o, dtype=np.float32)
    b_o = np.asarray(b_o, dtype=np.float32)
    import ml_dtypes

    scale = 1.0 / np.sqrt(np.float32(DH))

    # x^T per batch (shared between the two cores of a batch)
    xTs = [np.ascontiguousarray(x[b].T) for b in range(B)]

    # causal mask tile: keep iff q-local >= k-local (upper triangular w/ diag)
    tri1 = np.triu(np.ones((128, 128), np.float32))
    tri = np.concatenate([tri1, tri1], axis=1).astype(ml_dtypes.bfloat16)

    in_maps = []
    for c in range(N_CORES):
        b, g = divmod(c, 2)
        h0 = g * HPC
        qcols = slice(h0 * DH, h0 * DH + DPC)
        kcols = slice(D + h0 * DH, D + h0 * DH + DPC)
        vcols = slice(2 * D + h0 * DH, 2 * D + h0 * DH + DPC)
        wqk = np.concatenate(
            [W_qkv[:, qcols] * scale, W_qkv[:, kcols]], axis=1
        ).astype(np.float32)
        bqk = np.concatenate(
            [b_qkv[qcols] * scale, b_qkv[kcols]]
        ).astype(np.float32)
        in_maps.append(
            {
                "xT": xTs[b],
                "wqk": np.ascontiguousarray(wqk),
                "wv": np.ascontiguousarray(W_qkv[:, vcols]),
                "wo": np.ascontiguousarray(W_o[g * DPC : (g + 1) * DPC, :]),
                "bqk": np.ascontiguousarray(bqk.reshape(8, 128).T),
                "bv": np.ascontiguousarray(b_qkv[vcols].reshape(1, DPC)),
                "trimask": tri,
                "ones": np.ones((1, 128), np.float32),
            }
        )

    nc = _get_nc()
    global LAST_EXEC_NS
    res = run_bass_kernel_spmd(nc, in_maps, list(range(N_CORES)), trace=TRACE)
    LAST_EXEC_NS = res.exec_time_ns
    LAST_RES = globals().setdefault("_LAST_RES", None)
    globals()["_LAST_RES"] = res
    parts = [res.results[c]["out"] for c in range(N_CORES)]
    out = np.empty((B, T, D), np.float32)
    for b in range(B):
        out[b] = parts[2 * b] + parts[2 * b + 1] + b_o[None, :]
    return out
